# revision 11
# baseline (speedup 1.0000x reference)
"""Expert-parallel MoE SwiGLU kernel for one TRN2 chip (8 NeuronCores).

Problem: out[n] = sum_k w[n,k] * FFN_{idx[n,k]}(x[n]) with E=8 experts,
top-2 routing, H=1024, I=4096, N=2048 tokens.

Strategy: one expert per core. Tokens are routed (gathered) per expert on
the host, each core runs the three bf16 matmuls of its expert's SwiGLU FFN
(silu(x@w1) * (x@w3)) @ w2 over its token batch entirely transposed
(tokens along the PE moving/free dim), and the host scatter-adds the
returned per-expert outputs with the routing weights. Tokens whose two
routing slots hit the SAME expert are deduplicated on the host (weights
summed, FFN computed once). Expert token counts above the per-core
capacity CAP spill to a small host-side f32 pass so the device runs a
single full-width chunk.

Schedule notes (from NTFF profile analysis): the kernel is tensor-bound
(768 bf16 matmuls of C moving columns at ~0.42 ns/col; fp8 is ruled out
by the 2e-2 relative-error budget — measured 5.7% rel err). Startup: the
engine preamble runs ~7.3us, then the DMA rings start (sync ring's first
bytes ~0.8us after trigger, scalar ~3.2us, gpsimd ~4.3us). The
startup-critical set (x, ii=0 weights) is split so the first chain can
start ~12.9us: x (1MB) rides sync, w3[ii=0] (256KB) leads the scalar
ring, w1[ii=0] follows x on sync, and the ii=0 u-chain (w3) runs before
the g-chain. Warmup matmuls ramp the HAM clock and keep the PE busy
until then. The w13 weight stream arrives as 1MB pair tiles rotated
across ALL THREE rings (sync is otherwise idle mid-kernel; two rings
alone run ~149GB/s vs ~145GB/s needed and cost a matmul slot every
~10us), w2 likewise in Phase B. y is returned as bf16 (halves the store
traffic; +0.4% quadrature error, well within budget) and the final
output chunk is split into pipelined pieces on both hardware DGE rings
to shorten the kernel tail.
"""

import sys

for _p in ("/opt/trn_rl_repo", "/opt/pypackages"):
    if _p not in sys.path:
        sys.path.insert(0, _p)

import numpy as np
import ml_dtypes

import concourse.tile as tile
from concourse import bacc, mybir
from concourse.bass_utils import run_bass_kernel_spmd

P = 128
H = 1024
I = 4096
KH = H // P    # 8 contraction subtiles for the first matmuls
II = I // P    # 32 intermediate subtiles / contraction subtiles for w2
CAP = 480      # per-core token capacity (single PE moving chunk)
# PE warmup matmuls: ramp the HAM clock AND keep the tensor engine busy
# until the startup-critical DMAs are fully resident. Overshoot costs
# ~107ns/matmul; undershoot risks a HAM re-throttle that halves the
# early real matmul rate.
N_WARM = 36
W_COLS = 256   # warmup matmul moving width (finer tail granularity)
TAIL = 240     # final output piece width (short kernel tail). Keep the
               # final chain's moving width >= ~230 cols: below that the
               # 32-matmul chain goes LDWEIGHTS-bound (~97ns/matmul floor)
               # and the split costs more PE time than the tail it saves.

BF16 = mybir.dt.bfloat16
F32 = mybir.dt.float32


def _build(C):
    """One-expert SwiGLU FFN over C tokens (C <= 512), transposed layout.

    DRAM inputs (per core):
      xg    [P, KH, C]          bf16  x^T: [hp, kh, c] = x[tok c, kh*P+hp]
      w10t  [P, KH, P]          bf16  w1 ii=0 block: [hp, kh, m]
      w30t  [P, KH, P]          bf16  w3 ii=0 block
      w13st [3, P, 2, KH, P]    bf16  ii=1..3 singles (sync ring):
            [i, hp, 0, kh, m] = w1[kh*P+hp, (i+1)*P+m], [_,1,_] = w3
      w13pt [14, P, 2, 2, KH, P] bf16, pair-major ii blocks (ii=4..31):
            [j, hp, a, 0, kh, m] = w1[kh*P+hp, (2j+4+a)*P+m], [...,1,...] = w3
      w2t   [KH/2, P, 2, II, P] bf16, pair-major hh blocks:
            [q, ip, b, ik, m] = w2[ik*P+ip, (2q+b)*P+m]
    Output:
      yt    [KH/2, P, 2, C]     bf16  y^T, pair-major output subtiles
    """
    assert C <= 512
    nc = bacc.Bacc("TRN2", target_bir_lowering=False, debug=False)
    xg = nc.dram_tensor("xg", [P, KH, C], BF16, kind="ExternalInput")
    w10t = nc.dram_tensor("w10t", [P, KH, P], BF16, kind="ExternalInput")
    w30t = nc.dram_tensor("w30t", [P, KH, P], BF16, kind="ExternalInput")
    w13st = nc.dram_tensor(
        "w13st", [3, P, 2, KH, P], BF16, kind="ExternalInput"
    )
    w13pt = nc.dram_tensor(
        "w13pt", [(II - 4) // 2, P, 2, 2, KH, P], BF16, kind="ExternalInput"
    )
    w2t = nc.dram_tensor(
        "w2t", [KH // 2, P, 2, II, P], BF16, kind="ExternalInput"
    )
    yt = nc.dram_tensor("yt", [KH // 2, P, 2, C], BF16, kind="ExternalOutput")

    with tile.TileContext(nc) as tc:
        with (
            tc.tile_pool(name="xp", bufs=1) as xp,
            tc.tile_pool(name="pp", bufs=1) as pp,
            tc.tile_pool(name="wp", bufs=8) as wp,
            tc.tile_pool(name="w2p", bufs=2) as w2p,
            tc.tile_pool(name="gp", bufs=4) as gp,
            tc.tile_pool(name="yp", bufs=3) as yp,
            tc.tile_pool(name="warm", bufs=1) as warm,
            tc.tile_pool(name="psA", bufs=3, space="PSUM") as psA,
            tc.tile_pool(name="psB", bufs=2, space="PSUM") as psB,
        ):
            # Startup-critical loads. The sync ring's first bytes land
            # ~0.8us after its trigger, the scalar ring's ~3.2us after,
            # gpsimd's ~4.3us; triggers issue ~7.3us (end of engine
            # preamble). Empirical HBM arbitration: gpsimd's software ring
            # pulls ~190GB/s, scalar ~60-160, and the sync ring COLLAPSES
            # to ~25GB/s while the other two are active (it runs ~290GB/s
            # alone). So: x (1MB, 8KB lines) rides sync solo (lands
            # ~12.4us), w3[ii=0] leads the fast gpsimd ring (~12.9us)
            # while w1[ii=0] rides scalar IN PARALLEL (~13.2us), and sync
            # carries nothing else until the other rings drain. The ii=0
            # u-chain (w3) runs before the g-chain.
            # high_priority pins these triggers at the head of their
            # engine queues, ahead of the ACT_TABLE_LOAD the framework
            # schedules on the scalar queue.
            with tc.high_priority():
                xsb = xp.tile([P, KH, C], BF16)
                nc.sync.dma_start(xsb[:], xg[:])
                w30sb = wp.tile([P, KH, P], BF16, tag="w0", bufs=2)
                nc.scalar.dma_start(w30sb[:], w30t[:])
                w10sb = wp.tile([P, KH, P], BF16, tag="w0", bufs=2)
                nc.gpsimd.dma_start(w10sb[:], w10t[:])

            def xh(kh):
                return xsb[:, kh, :]

            # PE warmup: ramp the tensor engine to high-activity clock while
            # the input DMAs are in flight. Reads a zeroed tile, result is
            # never consumed.
            wtile = warm.tile([P, W_COLS], BF16)
            nc.vector.memset(wtile[:], 0.0)
            # Shares the Phase B psum pool (tag "py"): warmup is long done
            # before Phase B allocates its first chain psum.
            wps = psB.tile([P, W_COLS], F32, tag="py")
            for i in range(N_WARM):
                nc.tensor.matmul(
                    wps, wtile[:, :P], wtile[:], start=(i == 0),
                    stop=(i == N_WARM - 1),
                )

            psb = pp.tile([P, II, C], BF16)

            # Phase A: h1 = silu(x@w1), h3 = x@w3, p = h1*h3 (all transposed)
            # w13 stream: ii=1,2 singles lead the (fast) gpsimd ring, ii=3
            # on scalar behind the ii=0 blocks; ii>=4 as 1MB pair tiles
            # (one DMA per two ii — fewer triggers and semaphores) rotated
            # gpsimd/scalar/sync (sync only gets bandwidth once the other
            # rings drain to their mid-kernel just-in-time trickle, so its
            # first pair is j=2, needed last among the first rotation).
            wpair = None
            for ii in range(II):
                if ii == 0:
                    wsel = lambda half, kh: (
                        w10sb[:, kh, :] if half == 0 else w30sb[:, kh, :]
                    )
                elif ii in (1, 2, 3):
                    # All three singles ride sync right behind x: the sync
                    # ring greedily takes ~200GB/s once queued (observed),
                    # delivering ii1/ii2/ii3 by ~14.5/17/19us — well ahead
                    # of their chain need-times. scalar/gpsimd early carry
                    # only the small ii=0 blocks, minimizing contention.
                    wsb = wp.tile([P, 2, KH, P], BF16, tag="w13", bufs=3)
                    nc.sync.dma_start(wsb[:], w13st[ii - 1])
                    wsel = lambda half, kh, t=wsb: t[:, half, kh, :]
                elif (ii - 4) % 2 == 0:
                    j = (ii - 4) // 2
                    wpair = wp.tile([P, 2, 2, KH, P], BF16, tag="w13p", bufs=3)
                    if j < 2:
                        # behind the ii=0 blocks on the hw rings; these are
                        # not needed until ~26/30us.
                        eng = (nc.scalar, nc.gpsimd)[j]
                    else:
                        eng = (nc.scalar, nc.gpsimd, nc.sync)[(j - 2) % 3]
                    eng.dma_start(wpair[:], w13pt[j])
                    wsel = lambda half, kh, t=wpair: t[:, 0, half, kh, :]
                else:
                    wsel = lambda half, kh, t=wpair: t[:, 1, half, kh, :]
                pg = psA.tile([P, C], F32, tag="pg")
                pu = psA.tile([P, C], F32, tag="pu")
                # u-chain (w3, arrives first on its ring) before g-chain.
                halves = (1, 0) if ii < 4 else (0, 1)
                for half in halves:
                    ps = pg if half == 0 else pu
                    for kh in range(KH):
                        nc.tensor.matmul(
                            ps,
                            wsel(half, kh),
                            xh(kh),
                            start=(kh == 0),
                            stop=(kh == KH - 1),
                        )
                gs = gp.tile([P, C], BF16, tag="g")
                nc.scalar.activation(gs, pg, mybir.ActivationFunctionType.Silu)
                nc.vector.tensor_tensor(
                    psb[:, ii, :], gs, pu, mybir.AluOpType.mult
                )

            # Phase B: y = p @ w2 (transposed: yT = w2T-contraction over I).
            # w2 arrives as 1MB pair tiles (hh 2q, 2q+1 together) rotated
            # across gpsimd/sync/scalar (gpsimd's trigger stream runs ahead
            # of compute, so q=0 prefetches early) and y goes out as bf16
            # pair stores. The last hh is split column-wise so its first
            # piece's copy+DMA overlaps the final piece's matmuls (shorter
            # kernel tail), with the final small piece on the scalar ring.
            w2sb = yd = None
            for hh in range(KH):
                q, b = divmod(hh, 2)
                if b == 0:
                    w2sb = w2p.tile([P, 2, II, P], BF16, tag="w2")
                    # scalar/gpsimd only: their trigger streams run ahead of
                    # compute, while the sync queue is serialized behind the
                    # data-dependent y stores.
                    eng = (nc.gpsimd, nc.scalar, nc.gpsimd, nc.scalar)[q]
                    eng.dma_start(w2sb[:], w2t[q])
                    yd = yp.tile([P, 2, C], BF16, tag="y2")
                halves = [(0, C)] if hh < KH - 1 or C <= TAIL else [
                    (0, C - TAIL), (C - TAIL, TAIL),
                ]
                for hi, (c0, cc) in enumerate(halves):
                    py = psB.tile([P, cc], F32, tag="py")
                    for ik in range(II):
                        nc.tensor.matmul(
                            py,
                            w2sb[:, b, ik, :],
                            psb[:, ik, c0 : c0 + cc],
                            start=(ik == 0),
                            stop=(ik == II - 1),
                        )
                    # DVE copies keep the COPY activation table off the
                    # scalar queue (its ACT_TABLE_LOAD would delay the scalar
                    # DMA ring's startup-critical triggers by ~1.3us).
                    if hh < KH - 1 or hi == 0:
                        nc.vector.tensor_copy(yd[:, b, c0 : c0 + cc], py)
                        if b == 1 and hh < KH - 1:
                            nc.sync.dma_start(yt[q], yd[:])
                        elif hh == KH - 1:
                            # penultimate store: hh=6 whole + hh=7 first
                            # piece, one trigger
                            nc.sync.dma_start(
                                yt[q, :, 0, :], yd[:, 0, :]
                            )
                            nc.sync.dma_start(
                                yt[q, :, 1, c0 : c0 + cc],
                                yd[:, 1, c0 : c0 + cc],
                            )
                    else:
                        # Final piece on the other hardware DGE ring,
                        # pipelined behind the first piece's store.
                        yb = yp.tile([P, cc], BF16, tag="y")
                        nc.vector.tensor_copy(yb, py)
                        nc.scalar.dma_start(
                            yt[q, :, 1, c0 : c0 + cc], yb[:]
                        )

    nc.compile()
    return nc


_PROGRAM_CACHE = {}


def _host_swiglu(x, w1e, w2e, w3e):
    g = x @ w1e
    u = x @ w3e
    g = g / (1.0 + np.exp(-g))
    return (g * u) @ w2e


def kernel(x, expert_indices, expert_weights, w1, w2, w3):
    x = np.asarray(x, dtype=np.float32)
    idx = np.asarray(expert_indices)
    wts = np.asarray(expert_weights, dtype=np.float32)
    w1 = np.asarray(w1, dtype=np.float32)
    w2 = np.asarray(w2, dtype=np.float32)
    w3 = np.asarray(w3, dtype=np.float32)
    N = x.shape[0]
    E = w1.shape[0]
    K = idx.shape[1]
    bf16 = ml_dtypes.bfloat16

    # host-side routing with dedup: a token whose routing slots both hit
    # expert e is computed once with the slot weights summed (exact:
    # (w0+w1)*FFN = w0*FFN + w1*FFN). Tokens beyond CAP spill to the host
    # f32 path (tiny tail, keeps device at one full-width PE chunk).
    toks, tokw, spill_toks, spill_w = [], [], [], []
    for e in range(E):
        hit = idx == e  # [N, K]
        rows = np.nonzero(hit.any(axis=1))[0]
        w_e = (wts[rows] * hit[rows]).sum(axis=1)
        toks.append(rows[:CAP])
        tokw.append(w_e[:CAP])
        spill_toks.append(rows[CAP:])
        spill_w.append(w_e[CAP:])
    C = max(16, max(len(t) for t in toks))
    C = ((C + 7) // 8) * 8

    if C not in _PROGRAM_CACHE:
        _PROGRAM_CACHE[C] = _build(C)
    nc = _PROGRAM_CACHE[C]

    in_maps = []
    for e in range(E):
        xt = np.zeros((C, H), dtype=np.float32)
        if len(toks[e]):
            xt[: len(toks[e])] = x[toks[e]]
        # [C, H] -> [hp, kh, c]
        xge = xt.T.reshape(KH, P, C).transpose(1, 0, 2)
        # w1/w3 [H, I] -> [ii, hp, {w1,w3}, kh, m]
        w13 = np.stack(
            [
                w1[e].reshape(KH, P, II, P).transpose(2, 1, 0, 3),
                w3[e].reshape(KH, P, II, P).transpose(2, 1, 0, 3),
            ],
            axis=2,
        )  # [II, P, 2, KH, P]
        w13 = w13.astype(bf16)
        # pair-major pairs for ii>=4: [II/2-2, P, 2, 2, KH, P]
        w13p = np.ascontiguousarray(
            w13[4:].reshape((II - 4) // 2, 2, P, 2, KH, P).swapaxes(1, 2)
        )
        in_maps.append(
            {
                "xg": np.ascontiguousarray(xge.astype(bf16)),
                "w10t": np.ascontiguousarray(w13[0, :, 0]),
                "w30t": np.ascontiguousarray(w13[0, :, 1]),
                "w13st": np.ascontiguousarray(w13[1:4]),
                "w13pt": w13p,
                "w2t": np.ascontiguousarray(
                    w2[e].reshape(II, P, KH, P).transpose(2, 1, 0, 3)
                    .reshape(KH // 2, 2, P, II, P).swapaxes(1, 2).astype(bf16)
                ),
            }
        )

    res = run_bass_kernel_spmd(nc, in_maps, core_ids=list(range(E)))

    out = np.zeros((N, H), dtype=np.float32)
    for e in range(E):
        cnt = len(toks[e])
        if cnt:
            y = (
                res.results[e]["yt"]
                .astype(np.float32)
                .reshape(KH // 2, P, 2, C)
                .swapaxes(1, 2)
                .reshape(H, C)
                .T[:cnt]
            )
            np.add.at(out, toks[e], y * tokw[e][:, None])
        if len(spill_toks[e]):
            ys = _host_swiglu(x[spill_toks[e]], w1[e], w2[e], w3[e])
            np.add.at(out, spill_toks[e], ys * spill_w[e][:, None])
    return out


# revision 12
# speedup vs baseline: 1.0771x; 1.0771x over previous
"""Expert-parallel MoE SwiGLU kernel for one TRN2 chip (8 NeuronCores).

Problem: out[n] = sum_k w[n,k] * FFN_{idx[n,k]}(x[n]) with E=8 experts,
top-2 routing, H=1024, I=4096, N=2048 tokens.

Strategy: one expert per core. Tokens are routed (gathered) per expert on
the host, each core runs the three bf16 matmuls of its expert's SwiGLU FFN
(silu(x@w1) * (x@w3)) @ w2 over its token batch entirely transposed
(tokens along the PE moving/free dim), and the host scatter-adds the
returned per-expert outputs with the routing weights. Tokens whose two
routing slots hit the SAME expert are deduplicated on the host (weights
summed, FFN computed once). Expert token counts above the per-core
capacity CAP spill to a small host-side f32 pass so the device runs a
single full-width chunk.

Schedule notes (from NTFF profile analysis): the kernel is tensor-bound
(768 bf16 matmuls of C moving columns at ~0.42 ns/col; fp8 is ruled out
by the 2e-2 relative-error budget — measured 5.7% rel err). Startup: the
engine preamble runs ~7.3us, then the DMA rings start (sync ring's first
bytes ~0.8us after trigger, scalar ~3.2us, gpsimd ~4.3us). The
startup-critical set (x, ii=0 weights) is split so the first chain can
start ~12.9us: x (1MB) rides sync, w3[ii=0] (256KB) leads the scalar
ring, w1[ii=0] follows x on sync, and the ii=0 u-chain (w3) runs before
the g-chain. Warmup matmuls ramp the HAM clock and keep the PE busy
until then. The w13 weight stream arrives as 1MB pair tiles rotated
across ALL THREE rings (sync is otherwise idle mid-kernel; two rings
alone run ~149GB/s vs ~145GB/s needed and cost a matmul slot every
~10us), w2 likewise in Phase B. y is returned as bf16 (halves the store
traffic; +0.4% quadrature error, well within budget) and the final
output chunk is split into pipelined pieces on both hardware DGE rings
to shorten the kernel tail.
"""

import sys

for _p in ("/opt/trn_rl_repo", "/opt/pypackages"):
    if _p not in sys.path:
        sys.path.insert(0, _p)

import numpy as np
import ml_dtypes

import concourse.tile as tile
from concourse import bacc, mybir
from concourse.bass_utils import run_bass_kernel_spmd

P = 128
H = 1024
I = 4096
KH = H // P    # 8 contraction subtiles for the first matmuls
II = I // P    # 32 intermediate subtiles / contraction subtiles for w2
CAP = 480      # per-core token capacity (single PE moving chunk)
# PE warmup matmuls: ramp the HAM clock AND keep the tensor engine busy
# until the startup-critical DMAs are fully resident. Overshoot costs
# ~107ns/matmul; undershoot risks a HAM re-throttle that halves the
# early real matmul rate.
N_WARM = 36
W_COLS = 256   # warmup matmul moving width (finer tail granularity)
TAIL = 240     # final output piece width (short kernel tail). Keep the
               # final chain's moving width >= ~230 cols: below that the
               # 32-matmul chain goes LDWEIGHTS-bound (~97ns/matmul floor)
               # and the split costs more PE time than the tail it saves.

BF16 = mybir.dt.bfloat16
F32 = mybir.dt.float32


def _build(C):
    """One-expert SwiGLU FFN over C tokens (C <= 512), transposed layout.

    DRAM inputs (per core):
      xg    [P, KH, C]          bf16  x^T: [hp, kh, c] = x[tok c, kh*P+hp]
      w10t  [P, KH, P]          bf16  w1 ii=0 block: [hp, kh, m]
      w30t  [P, KH, P]          bf16  w3 ii=0 block
      w13st [3, P, 2, KH, P]    bf16  ii=1..3 singles (sync ring):
            [i, hp, 0, kh, m] = w1[kh*P+hp, (i+1)*P+m], [_,1,_] = w3
      w13pt [14, P, 2, 2, KH, P] bf16, pair-major ii blocks (ii=4..31):
            [j, hp, a, 0, kh, m] = w1[kh*P+hp, (2j+4+a)*P+m], [...,1,...] = w3
      w2t   [KH/2, P, 2, II, P] bf16, pair-major hh blocks:
            [q, ip, b, ik, m] = w2[ik*P+ip, (2q+b)*P+m]
    Output:
      yt    [KH/2, P, 2, C]     bf16  y^T, pair-major output subtiles
    """
    assert C <= 512
    nc = bacc.Bacc("TRN2", target_bir_lowering=False, debug=False)
    xg = nc.dram_tensor("xg", [P, KH, C], BF16, kind="ExternalInput")
    w10t = nc.dram_tensor("w10t", [P, KH, P], BF16, kind="ExternalInput")
    w30t = nc.dram_tensor("w30t", [P, KH, P], BF16, kind="ExternalInput")
    w13st = nc.dram_tensor(
        "w13st", [3, P, 2, KH, P], BF16, kind="ExternalInput"
    )
    w13pt = nc.dram_tensor(
        "w13pt", [(II - 4) // 2, P, 2, 2, KH, P], BF16, kind="ExternalInput"
    )
    w2t = nc.dram_tensor(
        "w2t", [KH // 2, P, 2, II, P], BF16, kind="ExternalInput"
    )
    yt = nc.dram_tensor("yt", [KH // 2, P, 2, C], BF16, kind="ExternalOutput")

    with tile.TileContext(nc) as tc:
        with (
            tc.tile_pool(name="xp", bufs=1) as xp,
            tc.tile_pool(name="pp", bufs=1) as pp,
            tc.tile_pool(name="wp", bufs=8) as wp,
            tc.tile_pool(name="w2p", bufs=2) as w2p,
            tc.tile_pool(name="gp", bufs=4) as gp,
            tc.tile_pool(name="yp", bufs=3) as yp,
            tc.tile_pool(name="warm", bufs=1) as warm,
            tc.tile_pool(name="psA", bufs=3, space="PSUM") as psA,
            tc.tile_pool(name="psB", bufs=2, space="PSUM") as psB,
        ):
            # Startup-critical loads. The sync ring's first bytes land
            # ~0.8us after its trigger, the scalar ring's ~3.2us after,
            # gpsimd's ~4.3us; triggers issue ~7.3us (end of engine
            # preamble). Empirical HBM arbitration: gpsimd's software ring
            # pulls ~190GB/s, scalar ~60-160, and the sync ring COLLAPSES
            # to ~25GB/s while the other two are active (it runs ~290GB/s
            # alone). So: x (1MB, 8KB lines) rides sync solo (lands
            # ~12.4us), w3[ii=0] leads the fast gpsimd ring (~12.9us)
            # while w1[ii=0] rides scalar IN PARALLEL (~13.2us), and sync
            # carries nothing else until the other rings drain. The ii=0
            # u-chain (w3) runs before the g-chain.
            # high_priority pins these triggers at the head of their
            # engine queues, ahead of the ACT_TABLE_LOAD the framework
            # schedules on the scalar queue.
            with tc.high_priority():
                xsb = xp.tile([P, KH, C], BF16)
                nc.sync.dma_start(xsb[:], xg[:])
                w30sb = wp.tile([P, KH, P], BF16, tag="w0", bufs=2)
                nc.scalar.dma_start(w30sb[:], w30t[:])
                w10sb = wp.tile([P, KH, P], BF16, tag="w0", bufs=2)
                nc.gpsimd.dma_start(w10sb[:], w10t[:])

            def xh(kh):
                return xsb[:, kh, :]

            # PE warmup: ramp the tensor engine to high-activity clock while
            # the input DMAs are in flight. Reads a zeroed tile, result is
            # never consumed.
            wtile = warm.tile([P, W_COLS], BF16)
            nc.vector.memset(wtile[:], 0.0)
            # Shares the Phase B psum pool (tag "py"): warmup is long done
            # before Phase B allocates its first chain psum.
            wps = psB.tile([P, W_COLS], F32, tag="py")
            for i in range(N_WARM):
                nc.tensor.matmul(
                    wps, wtile[:, :P], wtile[:], start=(i == 0),
                    stop=(i == N_WARM - 1),
                )

            psb = pp.tile([P, II, C], BF16)

            # Phase A: h1 = silu(x@w1), h3 = x@w3, p = h1*h3 (all transposed)
            # w13 stream: ii=1,2 singles lead the (fast) gpsimd ring, ii=3
            # on scalar behind the ii=0 blocks; ii>=4 as 1MB pair tiles
            # (one DMA per two ii — fewer triggers and semaphores) rotated
            # gpsimd/scalar/sync (sync only gets bandwidth once the other
            # rings drain to their mid-kernel just-in-time trickle, so its
            # first pair is j=2, needed last among the first rotation).
            wpair = None
            for ii in range(II):
                if ii == 0:
                    wsel = lambda half, kh: (
                        w10sb[:, kh, :] if half == 0 else w30sb[:, kh, :]
                    )
                elif ii in (1, 2, 3):
                    # Singles ride the two early-capable rings (gpsimd is
                    # the strongest, scalar next; sync starves once others
                    # are active). Crucially they share the PAIRS' pool tag
                    # (bufs=3): pair j0 then allocates the ii=1 single's
                    # buffer and its trigger WAITS until the ii=1 chains
                    # complete — pair prefetch becomes compute-paced, so
                    # the early rings carry only ~3MB of startup-critical
                    # data instead of flooding HBM with 1MB pair tiles.
                    wsb = wp.tile([P, 2, KH, P], BF16, tag="w13p", bufs=3)
                    eng = nc.scalar if ii == 3 else nc.gpsimd
                    eng.dma_start(wsb[:], w13st[ii - 1])
                    wsel = lambda half, kh, t=wsb: t[:, half, kh, :]
                elif (ii - 4) % 2 == 0:
                    j = (ii - 4) // 2
                    wpair = wp.tile([P, 2, 2, KH, P], BF16, tag="w13p", bufs=3)
                    eng = (nc.gpsimd, nc.scalar, nc.sync)[j % 3]
                    eng.dma_start(wpair[:], w13pt[j])
                    wsel = lambda half, kh, t=wpair: t[:, 0, half, kh, :]
                else:
                    wsel = lambda half, kh, t=wpair: t[:, 1, half, kh, :]
                pg = psA.tile([P, C], F32, tag="pg")
                pu = psA.tile([P, C], F32, tag="pu")
                # u-chain (w3, arrives first on its ring) before g-chain.
                halves = (1, 0) if ii < 4 else (0, 1)
                for half in halves:
                    ps = pg if half == 0 else pu
                    for kh in range(KH):
                        nc.tensor.matmul(
                            ps,
                            wsel(half, kh),
                            xh(kh),
                            start=(kh == 0),
                            stop=(kh == KH - 1),
                        )
                gs = gp.tile([P, C], BF16, tag="g")
                nc.scalar.activation(gs, pg, mybir.ActivationFunctionType.Silu)
                nc.vector.tensor_tensor(
                    psb[:, ii, :], gs, pu, mybir.AluOpType.mult
                )

            # Phase B: y = p @ w2 (transposed: yT = w2T-contraction over I).
            # w2 arrives as 1MB pair tiles (hh 2q, 2q+1 together) rotated
            # across gpsimd/sync/scalar (gpsimd's trigger stream runs ahead
            # of compute, so q=0 prefetches early) and y goes out as bf16
            # pair stores. The last hh is split column-wise so its first
            # piece's copy+DMA overlaps the final piece's matmuls (shorter
            # kernel tail), with the final small piece on the scalar ring.
            w2sb = yd = None
            for hh in range(KH):
                q, b = divmod(hh, 2)
                if b == 0:
                    w2sb = w2p.tile([P, 2, II, P], BF16, tag="w2")
                    # scalar/gpsimd only: their trigger streams run ahead of
                    # compute, while the sync queue is serialized behind the
                    # data-dependent y stores.
                    eng = (nc.gpsimd, nc.scalar, nc.gpsimd, nc.scalar)[q]
                    eng.dma_start(w2sb[:], w2t[q])
                    yd = yp.tile([P, 2, C], BF16, tag="y2")
                halves = [(0, C)] if hh < KH - 1 or C <= TAIL else [
                    (0, C - TAIL), (C - TAIL, TAIL),
                ]
                for hi, (c0, cc) in enumerate(halves):
                    py = psB.tile([P, cc], F32, tag="py")
                    for ik in range(II):
                        nc.tensor.matmul(
                            py,
                            w2sb[:, b, ik, :],
                            psb[:, ik, c0 : c0 + cc],
                            start=(ik == 0),
                            stop=(ik == II - 1),
                        )
                    # DVE copies keep the COPY activation table off the
                    # scalar queue (its ACT_TABLE_LOAD would delay the scalar
                    # DMA ring's startup-critical triggers by ~1.3us).
                    if hh < KH - 1 or hi == 0:
                        nc.vector.tensor_copy(yd[:, b, c0 : c0 + cc], py)
                        if b == 1 and hh < KH - 1:
                            nc.sync.dma_start(yt[q], yd[:])
                        elif hh == KH - 1:
                            # penultimate store: hh=6 whole + hh=7 first
                            # piece, one trigger
                            nc.sync.dma_start(
                                yt[q, :, 0, :], yd[:, 0, :]
                            )
                            nc.sync.dma_start(
                                yt[q, :, 1, c0 : c0 + cc],
                                yd[:, 1, c0 : c0 + cc],
                            )
                    else:
                        # Final piece on the other hardware DGE ring,
                        # pipelined behind the first piece's store.
                        yb = yp.tile([P, cc], BF16, tag="y")
                        nc.vector.tensor_copy(yb, py)
                        nc.scalar.dma_start(
                            yt[q, :, 1, c0 : c0 + cc], yb[:]
                        )

    nc.compile()
    return nc


_PROGRAM_CACHE = {}


def _host_swiglu(x, w1e, w2e, w3e):
    g = x @ w1e
    u = x @ w3e
    g = g / (1.0 + np.exp(-g))
    return (g * u) @ w2e


def kernel(x, expert_indices, expert_weights, w1, w2, w3):
    x = np.asarray(x, dtype=np.float32)
    idx = np.asarray(expert_indices)
    wts = np.asarray(expert_weights, dtype=np.float32)
    w1 = np.asarray(w1, dtype=np.float32)
    w2 = np.asarray(w2, dtype=np.float32)
    w3 = np.asarray(w3, dtype=np.float32)
    N = x.shape[0]
    E = w1.shape[0]
    K = idx.shape[1]
    bf16 = ml_dtypes.bfloat16

    # host-side routing with dedup: a token whose routing slots both hit
    # expert e is computed once with the slot weights summed (exact:
    # (w0+w1)*FFN = w0*FFN + w1*FFN). Tokens beyond CAP spill to the host
    # f32 path (tiny tail, keeps device at one full-width PE chunk).
    toks, tokw, spill_toks, spill_w = [], [], [], []
    for e in range(E):
        hit = idx == e  # [N, K]
        rows = np.nonzero(hit.any(axis=1))[0]
        w_e = (wts[rows] * hit[rows]).sum(axis=1)
        toks.append(rows[:CAP])
        tokw.append(w_e[:CAP])
        spill_toks.append(rows[CAP:])
        spill_w.append(w_e[CAP:])
    C = max(16, max(len(t) for t in toks))
    C = ((C + 7) // 8) * 8

    if C not in _PROGRAM_CACHE:
        _PROGRAM_CACHE[C] = _build(C)
    nc = _PROGRAM_CACHE[C]

    in_maps = []
    for e in range(E):
        xt = np.zeros((C, H), dtype=np.float32)
        if len(toks[e]):
            xt[: len(toks[e])] = x[toks[e]]
        # [C, H] -> [hp, kh, c]
        xge = xt.T.reshape(KH, P, C).transpose(1, 0, 2)
        # w1/w3 [H, I] -> [ii, hp, {w1,w3}, kh, m]
        w13 = np.stack(
            [
                w1[e].reshape(KH, P, II, P).transpose(2, 1, 0, 3),
                w3[e].reshape(KH, P, II, P).transpose(2, 1, 0, 3),
            ],
            axis=2,
        )  # [II, P, 2, KH, P]
        w13 = w13.astype(bf16)
        # pair-major pairs for ii>=4: [II/2-2, P, 2, 2, KH, P]
        w13p = np.ascontiguousarray(
            w13[4:].reshape((II - 4) // 2, 2, P, 2, KH, P).swapaxes(1, 2)
        )
        in_maps.append(
            {
                "xg": np.ascontiguousarray(xge.astype(bf16)),
                "w10t": np.ascontiguousarray(w13[0, :, 0]),
                "w30t": np.ascontiguousarray(w13[0, :, 1]),
                "w13st": np.ascontiguousarray(w13[1:4]),
                "w13pt": w13p,
                "w2t": np.ascontiguousarray(
                    w2[e].reshape(II, P, KH, P).transpose(2, 1, 0, 3)
                    .reshape(KH // 2, 2, P, II, P).swapaxes(1, 2).astype(bf16)
                ),
            }
        )

    res = run_bass_kernel_spmd(nc, in_maps, core_ids=list(range(E)))

    out = np.zeros((N, H), dtype=np.float32)
    for e in range(E):
        cnt = len(toks[e])
        if cnt:
            y = (
                res.results[e]["yt"]
                .astype(np.float32)
                .reshape(KH // 2, P, 2, C)
                .swapaxes(1, 2)
                .reshape(H, C)
                .T[:cnt]
            )
            np.add.at(out, toks[e], y * tokw[e][:, None])
        if len(spill_toks[e]):
            ys = _host_swiglu(x[spill_toks[e]], w1[e], w2[e], w3[e])
            np.add.at(out, spill_toks[e], ys * spill_w[e][:, None])
    return out
